# revision 31
# baseline (speedup 1.0000x reference)
"""Trainium2 Bass kernel for the masked style-attention module.

Shapes (hardcoded): B=4, C_IN=256, C_KEY=448, H=W=64, N=4096.
Sharding: 8 cores = batch (4) x query-row half (2). Each core computes
out[b][:, h*2048:(h+1)*2048] for its (b, h), emitted in [n, c] layout
(host transposes during gather).

Math per core (rows n in its half, all m in 0..4095):
  G_aug[c,m] = Wg@skey + bg  (bias via ones row; row 448 = smi mask)
  Fq_aug[c,n] = Wf@ckey + bf (row 448 = cmneg = -1e15 * content_mask)
  Hv[m,c] = (Wh@style + bh)^T; hv2[m] = [Hv | Hv^2 | 1]  (513 cols)
  T[m,n] = sum_c G_aug[c,m] Fq_aug[c,n]   (mask folds in additively)
  P = exp(T)  (no row-max: |S| < ~40, fp32 psum / bf16 store safe)
  acc[n, 0:513] = sum_m P[m,n] * hv2[m, :]  -> [mean*R | m2*R | R]
  out[n,c] = sqrt(relu(m2-mean^2)) * mvn(content)[n,c] + mean[n,c]

All matmul operands are bf16 (fp32 PSUM accumulate); host pre-quantizes
and pre-rearranges inputs so every DMA is contiguous per partition.
mvn stats come from ones-row matmuls over contT (sum / sum-of-squares
along the partition axis), broadcast back with K=1 matmuls - no
partition transposes or gpsimd broadcasts anywhere.
"""

from contextlib import ExitStack

import numpy as np
import ml_dtypes

import concourse.bass as bass
from concourse import bacc
import concourse.mybir as mybir
import concourse.tile as tile
from concourse.bass_utils import run_bass_kernel_spmd

AF = mybir.ActivationFunctionType
ALU = mybir.AluOpType
F32 = mybir.dt.float32
BF16 = mybir.dt.bfloat16
U32 = mybir.dt.uint32

B, C_IN, C_KEY = 4, 256, 448
N = 4096
HALF = 2048
NEG = -1e15
EPS = 1e-5
CORR = N / (N - 1.0)  # unbiased-variance correction for mvn
MAGIC = 0x5F3759DF

KSZ = [128, 128, 128, 65]  # contraction tiles over 449 (= C_KEY + aug row)
CO448 = [(0, 128), (128, 128), (256, 128), (384, 64)]


def _build():
    nc = bacc.Bacc("TRN2", target_bir_lowering=False)

    skey = nc.dram_tensor("skey", [128, 4, N], BF16, kind="ExternalInput")
    wg = nc.dram_tensor("wg", [128, 4, 448], BF16, kind="ExternalInput")
    ckey = nc.dram_tensor("ckey", [128, 4, HALF], BF16, kind="ExternalInput")
    wf = nc.dram_tensor("wf", [128, 4, 448], BF16, kind="ExternalInput")
    styl = nc.dram_tensor("styl", [128, 2, N], BF16, kind="ExternalInput")
    wh = nc.dram_tensor("wh", [128, 2, 256], BF16, kind="ExternalInput")
    bh = nc.dram_tensor("bh", [1, 256], BF16, kind="ExternalInput")
    contT = nc.dram_tensor("contT", [128, 32, 256], BF16, kind="ExternalInput")
    smi = nc.dram_tensor("smi", [1, N], BF16, kind="ExternalInput")
    cmneg = nc.dram_tensor("cmneg", [1, HALF], BF16, kind="ExternalInput")
    out_d = nc.dram_tensor("out", [HALF, 256], F32, kind="ExternalOutput")

    with tile.TileContext(nc, pool_alloc_mode="queue") as tc:
        with tc.tile_pool(name="persist", bufs=1) as persist:
            g = [
                persist.tile([KSZ[k], N], BF16, tag=f"g{k}", name=f"g{k}")
                for k in range(4)
            ]
            fq = [
                persist.tile([KSZ[k], HALF], BF16, tag=f"fq{k}", name=f"fq{k}")
                for k in range(4)
            ]
            hvA = persist.tile([128, 32, 256], BF16, tag="hvA", name="hvA")
            hvB = persist.tile([128, 32, 257], BF16, tag="hvB", name="hvB")
            ct_sb = persist.tile([128, 32, 256], BF16, tag="ct_sb", name="ct_sb")
            aB = persist.tile([128, 256], F32, tag="aB", name="aB")
            bB = persist.tile([128, 256], F32, tag="bB", name="bB")
            ones1 = persist.tile([1, 128], F32, tag="ones1", name="ones1")
            nc.vector.memset(ones1, 1.0)
            onec = persist.tile([128, 1], BF16, tag="onec", name="onec")
            nc.vector.memset(onec, 1.0)
            nc.vector.memset(hvB[:, :, 256:257], 1.0)
            eps_t = persist.tile([1, 1], F32, tag="eps", name="eps")
            nc.vector.memset(eps_t, EPS)
            mgc = persist.tile([128, 512], U32, tag="mgc", name="mgc")
            nc.vector.memset(mgc, MAGIC)

            _pp = ExitStack()
            pw = _pp.enter_context(tc.tile_pool(name="pw", bufs=1))
            pin = _pp.enter_context(tc.tile_pool(name="pin", bufs=1))
            # DMA issue order = need order.
            wh_t = pw.tile([128, 3, 256], BF16, tag="wh_t", name="wh_t")
            nc.sync.dma_start(wh_t[:, 0:2, :], wh[:, :, :])
            nc.sync.dma_start(wh_t[0:1, 2, :], bh[:, :])
            st_sb = pin.tile([128, 3, N], BF16, tag="st_sb", name="st_sb")
            nc.vector.memset(st_sb[0:1, 2, :], 1.0)
            for q in range(4):
                qs = slice(q * (N // 4), (q + 1) * (N // 4))
                nc.sync.dma_start(st_sb[:, 0:2, qs], styl[:, :, qs])
            wg_t = pw.tile([128, 4, 448], BF16, tag="wg_t", name="wg_t")
            nc.sync.dma_start(wg_t, wg[:, :, :])
            sk_sb = pin.tile([128, 4, N], BF16, tag="sk_sb", name="sk_sb")
            nc.sync.dma_start(sk_sb[:, :, 0:HALF], skey[:, :, 0:HALF])
            nc.sync.dma_start(ct_sb, contT[:, :, :])
            nc.sync.dma_start(sk_sb[:, :, HALF:N], skey[:, :, HALF:N])
            ck_sb = pin.tile([128, 4, HALF], BF16, tag="ck_sb", name="ck_sb")
            nc.sync.dma_start(ck_sb, ckey[:, :, :])
            wf_t = pw.tile([128, 4, 448], BF16, tag="wf_t", name="wf_t")
            nc.sync.dma_start(wf_t, wf[:, :, :])

            # ---- Phase H: hv2[m] = [Wh@style+bh | (.)^2 | 1] ----
            # bias rides as a K=1 contraction row (st ones row x bh row), the
            # psum->sbuf copy runs on ACT and the square on gpsimd, so the
            # vector engine does no work here at all.
            with tc.tile_pool(name="psumH", bufs=8, space="PSUM") as pph:
                for mt in range(32):
                    ph = pph.tile([128, 256], F32, tag="ph", name="ph")
                    msl = slice(mt * 128, (mt + 1) * 128)
                    for k in range(2):
                        nc.tensor.matmul(
                            ph,
                            lhsT=st_sb[:, k, msl],
                            rhs=wh_t[:, k, :],
                            start=(k == 0),
                            stop=False,
                        )
                    nc.tensor.matmul(
                        ph,
                        lhsT=st_sb[0:1, 2, msl],
                        rhs=wh_t[0:1, 2, :],
                        start=False,
                        stop=True,
                    )
                    nc.scalar.copy(hvA[:, mt, :], ph)
                    nc.vector.tensor_mul(
                        hvB[:, mt, 0:256], hvA[:, mt, :], hvA[:, mt, :]
                    )

            # prime the Ln/Exp activation table set early (load ~2.7us) so it
            # doesn't land in the stats chain right before phase D
            prime = pw.tile([1, 1], F32, tag="prime", name="prime")
            nc.scalar.activation(prime, eps_t[0:1, 0:1], AF.Ln)

            # squares of contT for the sum-of-squares stats (idle gpsimd)
            sqT = pin.tile([128, 32, 256], BF16, tag="sqT", name="sqT")
            for t in range(32):
                nc.gpsimd.tensor_mul(sqT[:, t, :], ct_sb[:, t, :], ct_sb[:, t, :])

            # ---- Phase G: G_aug = Wg @ skey ----
            with tc.tile_pool(name="psumG", bufs=2, space="PSUM") as ppg:
                for half in range(2):
                    for co, (co0, cosz) in enumerate(CO448):
                        pgs = [
                            ppg.tile([128, 512], F32, tag=f"pg{ch}", name=f"pg{ch}")
                            for ch in range(4)
                        ]
                        for k in range(4):
                            for ch in range(4):
                                csl = slice(
                                    half * HALF + ch * 512,
                                    half * HALF + (ch + 1) * 512,
                                )
                                nc.tensor.matmul(
                                    pgs[ch][0:cosz, :],
                                    lhsT=wg_t[0 : KSZ[k], k, co0 : co0 + cosz],
                                    rhs=sk_sb[0 : KSZ[k], k, csl],
                                    start=(k == 0),
                                    stop=(k == 3),
                                )
                        for ch in range(4):
                            csl = slice(
                                half * HALF + ch * 512, half * HALF + (ch + 1) * 512
                            )
                            if (2 * half + co) % 2 == 0:
                                nc.scalar.copy(g[co][0:cosz, csl], pgs[ch][0:cosz, :])
                            else:
                                nc.vector.tensor_copy(
                                    g[co][0:cosz, csl], pgs[ch][0:cosz, :]
                                )
            nc.sync.dma_start(g[3][64:65, :], smi[:, :])

            # ---- Phase F: Fq_aug = Wf @ ckey ----
            with tc.tile_pool(name="psumF", bufs=2, space="PSUM") as ppf:
                for co, (co0, cosz) in enumerate(CO448):
                    pfs = [
                        ppf.tile([128, 512], F32, tag=f"pf{ch}", name=f"pf{ch}")
                        for ch in range(4)
                    ]
                    for k in range(4):
                        for ch in range(4):
                            csl = slice(ch * 512, (ch + 1) * 512)
                            nc.tensor.matmul(
                                pfs[ch][0:cosz, :],
                                lhsT=wf_t[0 : KSZ[k], k, co0 : co0 + cosz],
                                rhs=ck_sb[0 : KSZ[k], k, csl],
                                start=(k == 0),
                                stop=(k == 3),
                            )
                    for ch in range(4):
                        csl = slice(ch * 512, (ch + 1) * 512)
                        if co % 2 == 0:
                            nc.scalar.copy(fq[co][0:cosz, csl], pfs[ch][0:cosz, :])
                        else:
                            nc.vector.tensor_copy(
                                fq[co][0:cosz, csl], pfs[ch][0:cosz, :]
                            )
            nc.sync.dma_start(fq[3][64:65, :], cmneg[:, :])

            # ---- Phase A: mvn stats via ones-matmuls over contT ----
            pm = _pp.enter_context(tc.tile_pool(name="mvn", bufs=1))
            with tc.tile_pool(name="psumA", bufs=1, space="PSUM") as ppa:
                s_row = ppa.tile([1, 256], F32, tag="s_row", name="s_row")
                q_row = ppa.tile([1, 256], F32, tag="q_row", name="q_row")
                for t in range(32):
                    nc.tensor.matmul(
                        s_row,
                        lhsT=onec[:, 0:1],
                        rhs=ct_sb[:, t, :],
                        start=(t == 0),
                        stop=(t == 31),
                    )
                for t in range(32):
                    nc.tensor.matmul(
                        q_row,
                        lhsT=onec[:, 0:1],
                        rhs=sqT[:, t, :],
                        start=(t == 0),
                        stop=(t == 31),
                    )
                mean_r = pm.tile([1, 256], F32, tag="mean_r", name="mean_r")
                nc.vector.tensor_scalar(mean_r, s_row, 1.0 / N, None, ALU.mult)
                ex2_r = pm.tile([1, 256], F32, tag="ex2_r", name="ex2_r")
                nc.vector.tensor_scalar(ex2_r, q_row, 1.0 / N, None, ALU.mult)
                msq_r = pm.tile([1, 256], F32, tag="msq_r", name="msq_r")
                nc.vector.tensor_mul(msq_r, mean_r, mean_r)
                var_r = pm.tile([1, 256], F32, tag="var_r", name="var_r")
                nc.vector.scalar_tensor_tensor(
                    out=var_r,
                    in0=msq_r,
                    scalar=-1.0,
                    in1=ex2_r,
                    op0=ALU.mult,
                    op1=ALU.add,
                )
                lnv_r = pm.tile([1, 256], F32, tag="lnv_r", name="lnv_r")
                nc.scalar.activation(
                    lnv_r, var_r, AF.Ln, bias=eps_t[0:1, 0:1], scale=CORR
                )
                a_row = pm.tile([1, 256], F32, tag="a_row", name="a_row")
                nc.scalar.activation(a_row, lnv_r, AF.Exp, scale=-0.5)
                b_row = pm.tile([1, 256], F32, tag="b_row", name="b_row")
                nc.vector.scalar_tensor_tensor(
                    out=b_row,
                    in0=mean_r,
                    scalar=-1.0,
                    in1=a_row,
                    op0=ALU.mult,
                    op1=ALU.mult,
                )
                pab = ppa.tile([128, 256], F32, tag="pab", name="pab", bufs=2)
                nc.tensor.matmul(
                    pab, lhsT=ones1[0:1, :], rhs=a_row[0:1, :], start=True, stop=True
                )
                nc.vector.tensor_copy(aB, pab)
                pbb = ppa.tile([128, 256], F32, tag="pbb", name="pbb", bufs=2)
                nc.tensor.matmul(
                    pbb, lhsT=ones1[0:1, :], rhs=b_row[0:1, :], start=True, stop=True
                )
                nc.vector.tensor_copy(bB, pbb)
            _pp.close()  # free projection inputs + stats SBUF

            # ---- Phase D: attention, transposed layout ----
            _dpools = ExitStack()
            fin = _dpools.enter_context(tc.tile_pool(name="fin", bufs=1))
            ptp = _dpools.enter_context(tc.tile_pool(name="ptp", bufs=1))
            with (
                tc.tile_pool(name="ppt", bufs=4, space="PSUM") as ppt,
                tc.tile_pool(name="ppacc", bufs=1, space="PSUM") as ppacc,
            ):
                pend = {}

                def fin_release(st):
                    """Free the acc banks fast: reciprocal + ACT-side divide."""
                    accA, accB = st["accA"], st["accB"]
                    rinvs = []
                    for nt in range(2):
                        rinv = fin.tile([128, 1], F32, tag="rinv", name="rinv", bufs=4)
                        nc.vector.reciprocal(rinv, accB[nt][:, 256:257])
                        rinvs.append(rinv)
                    meanS = fin.tile(
                        [128, 2, 256], F32, tag="meanS", name="meanS", bufs=2
                    )
                    m2S = fin.tile([128, 2, 256], F32, tag="m2S", name="m2S", bufs=2)
                    for nt in range(2):
                        nc.scalar.mul(meanS[:, nt, :], accA[nt], rinvs[nt][:, 0:1])
                        nc.scalar.mul(
                            m2S[:, nt, :], accB[nt][:, 0:256], rinvs[nt][:, 0:1]
                        )
                    st["meanS"], st["m2S"] = meanS, m2S

                def _std_chain(e, mean_f, m2_f, mvn2_f, o2_f, w, mg):
                    """std + mvn + output on engine `e`, width-w tiles."""
                    msq = fin.tile([128, w], F32, tag="fw", name="msq", bufs=12)
                    e.tensor_mul(msq, mean_f, mean_f)
                    varp = fin.tile([128, w], F32, tag="fw", name="varp", bufs=12)
                    e.tensor_tensor(out=varp, in0=m2_f, in1=msq, op=ALU.subtract)
                    varc = fin.tile([128, w], F32, tag="fw", name="varc", bufs=12)
                    e.tensor_scalar_max(varc, varp, 1e-20)
                    sh = fin.tile([128, w], U32, tag="fw", name="sh", bufs=12)
                    e.tensor_scalar(
                        sh, varc.bitcast(U32), 1, None, ALU.logical_shift_right
                    )
                    y = fin.tile([128, w], F32, tag="fw", name="y0", bufs=12)
                    e.tensor_tensor(
                        out=y.bitcast(U32), in0=mg, in1=sh, op=ALU.subtract
                    )
                    ta = fin.tile([128, w], F32, tag="fw", name="ta", bufs=12)
                    e.tensor_mul(ta, y, y)
                    tb = fin.tile([128, w], F32, tag="fw", name="tb", bufs=12)
                    e.tensor_mul(tb, ta, varc)
                    tcn = fin.tile([128, w], F32, tag="fw", name="tcn", bufs=12)
                    e.tensor_scalar(tcn, tb, -0.5, 1.5, ALU.mult, ALU.add)
                    y2 = fin.tile([128, w], F32, tag="fw", name="y2", bufs=12)
                    e.tensor_mul(y2, y, tcn)
                    stdv = fin.tile([128, w], F32, tag="fw", name="stdv", bufs=12)
                    e.tensor_mul(stdv, varc, y2)
                    o1 = fin.tile([128, w], F32, tag="fw", name="o1", bufs=12)
                    e.tensor_mul(o1, mvn2_f, stdv)
                    e.tensor_add(o2_f, o1, mean_f)

                def fin_rest(st, split=False):
                    hc = st["hc"]
                    meanS, m2S, mvn2 = st["meanS"], st["m2S"], st["mvn2"]
                    o2 = fin.tile([128, 2, 256], F32, tag="o2", name="o2", bufs=2)
                    if not split:
                        _std_chain(
                            nc.vector,
                            meanS.rearrange("p a b -> p (a b)"),
                            m2S.rearrange("p a b -> p (a b)"),
                            mvn2.rearrange("p a b -> p (a b)"),
                            o2.rearrange("p a b -> p (a b)"),
                            512,
                            mgc,
                        )
                    else:
                        # final chunk: split across DVE and gpsimd to halve
                        # the serial tail
                        for nt, e in ((0, nc.vector), (1, nc.gpsimd)):
                            _std_chain(
                                e,
                                meanS[:, nt, :],
                                m2S[:, nt, :],
                                mvn2[:, nt, :],
                                o2[:, nt, :],
                                256,
                                mgc[:, 0:256],
                            )
                    nc.sync.dma_start(
                        out_d[hc * 256 : (hc + 1) * 256, :].rearrange(
                            "(t p) c -> p t c", p=128
                        ),
                        o2,
                    )

                for hc in range(8):
                    nsl = slice(hc * 256, (hc + 1) * 256)
                    accA = [
                        ppacc.tile([128, 256], F32, tag=f"accA{nt}", name=f"accA{nt}")
                        for nt in range(2)
                    ]
                    accB = [
                        ppacc.tile([128, 257], F32, tag=f"accB{nt}", name=f"accB{nt}")
                        for nt in range(2)
                    ]
                    mvn1 = fin.tile([128, 2, 256], F32, tag="mvn1", name="mvn1", bufs=2)
                    mvn2 = fin.tile([128, 2, 256], F32, tag="mvn2", name="mvn2", bufs=2)
                    for nt in range(2):
                        nc.vector.tensor_mul(
                            mvn1[:, nt, :], ct_sb[:, hc * 2 + nt, :], aB
                        )
                        nc.vector.tensor_add(mvn2[:, nt, :], mvn1[:, nt, :], bB)

                    def mm2(mt, pt_ap):
                        for nt in range(2):
                            lw = pt_ap[:, nt * 128 : (nt + 1) * 128]
                            nc.tensor.matmul(
                                accA[nt],
                                lhsT=lw,
                                rhs=hvA[:, mt, :],
                                start=(mt == 0),
                                stop=(mt == 31),
                            )
                            nc.tensor.matmul(
                                accB[nt],
                                lhsT=lw,
                                rhs=hvB[:, mt, :],
                                start=(mt == 0),
                                stop=(mt == 31),
                            )

                    prevs = []
                    for mt in range(32):
                        msl = slice(mt * 128, (mt + 1) * 128)
                        tp = ppt.tile([128, 256], F32, tag="tp", name="tp")
                        for k in range(4):
                            nc.tensor.matmul(
                                tp,
                                lhsT=g[k][0 : KSZ[k], msl],
                                rhs=fq[k][0 : KSZ[k], nsl],
                                start=(k == 0),
                                stop=(k == 3),
                            )
                        pt = ptp.tile([128, 256], BF16, tag="pt", name="pt", bufs=4)
                        nc.scalar.activation(pt, tp, AF.Exp)
                        if mt == 0 and pend:
                            fin_release(pend)
                        if mt == 2 and pend:
                            fin_rest(pend)
                            pend.clear()
                        prevs.append((mt, pt))
                        if len(prevs) > 2:  # 2-deep lag: exp never blocks PE
                            mm2(*prevs.pop(0))
                    for pr in prevs:
                        mm2(*pr)
                    pend = {"hc": hc, "accA": accA, "accB": accB, "mvn2": mvn2}
                fin_release(pend)
                fin_rest(pend)
            _dpools.close()
    nc.finalize()
    return nc


_nc_cache = None
last_results = None  # BassKernelResults of the most recent run (for test.py)


def _bf16(x):
    return np.asarray(x, dtype=ml_dtypes.bfloat16)


def _pad_k(a449):
    """[449, M] -> [128, 4, M] with k-tiles of 128/128/128/65 (pad to 128)."""
    m = a449.shape[1]
    outp = np.zeros((128, 4, m), a449.dtype)
    for k in range(3):
        outp[:, k, :] = a449[k * 128 : (k + 1) * 128, :]
    outp[0:65, 3, :] = a449[384:449, :]
    return np.ascontiguousarray(outp)


def _pad_k3(a257):
    """[257, M] -> [128, 3, M] with k-tiles of 128/128/1."""
    m = a257.shape[1]
    outp = np.zeros((128, 3, m), a257.dtype)
    for k in range(2):
        outp[:, k, :] = a257[k * 128 : (k + 1) * 128, :]
    outp[0:1, 2, :] = a257[256:257, :]
    return np.ascontiguousarray(outp)


def prepare_in_maps(
    content,
    style,
    content_key,
    style_key,
    content_mask,
    style_mask,
    Wf,
    bf,
    Wg,
    bg,
    Wh,
    bh,
):
    f32 = np.float32
    ones_n = np.ones((1, N), f32)
    ones_h = np.ones((1, HALF), f32)
    wgT = np.concatenate([np.asarray(Wg, f32).T, np.asarray(bg, f32)[None, :]], 0)
    wfT = np.concatenate([np.asarray(Wf, f32).T, np.asarray(bf, f32)[None, :]], 0)
    wg_in = _pad_k(_bf16(wgT))
    wf_in = _pad_k(_bf16(wfT))
    whT = np.asarray(Wh, f32).T.reshape(2, 128, 256).transpose(1, 0, 2)
    wh_in = np.ascontiguousarray(_bf16(whT))
    bh_in = np.ascontiguousarray(_bf16(np.asarray(bh, f32)[None, :]))

    in_maps = []
    for c in range(8):
        b, h = divmod(c, 2)
        hsl = slice(h * HALF, (h + 1) * HALF)
        sk = np.asarray(style_key[b], f32).reshape(C_KEY, N)
        ck = np.asarray(content_key[b], f32).reshape(C_KEY, N)[:, hsl]
        st = np.asarray(style[b], f32).reshape(C_IN, N)
        co = np.asarray(content[b], f32).reshape(C_IN, N)
        smi_in = (np.asarray(style_mask[b], np.int32).reshape(1, N) == 0).astype(f32)
        cm = np.asarray(content_mask[b], np.int32).reshape(N)[hsl]
        cmneg_in = ((cm != 0).astype(f32) * np.float32(NEG))[None, :]
        st_in = np.ascontiguousarray(_bf16(st).reshape(2, 128, N).transpose(1, 0, 2))
        osl = slice((1 - h) * HALF, (2 - h) * HALF)
        coT = np.concatenate([co[:, hsl].T, co[:, osl].T], 0)
        contT_in = _bf16(coT.reshape(32, 128, 256).transpose(1, 0, 2))
        in_maps.append(
            {
                "skey": _pad_k(_bf16(np.concatenate([sk, ones_n], 0))),
                "wg": wg_in,
                "ckey": _pad_k(_bf16(np.concatenate([ck, ones_h], 0))),
                "wf": wf_in,
                "styl": st_in,
                "wh": wh_in,
                "bh": bh_in,
                "contT": np.ascontiguousarray(contT_in),
                "smi": np.ascontiguousarray(_bf16(smi_in)),
                "cmneg": np.ascontiguousarray(_bf16(cmneg_in)),
            }
        )

    return in_maps


def get_nc():
    global _nc_cache
    if _nc_cache is None:
        _nc_cache = _build()
    return _nc_cache


def gather_output(outs):
    full = np.empty((B, C_IN, N), np.float32)
    for c in range(8):
        b, h = divmod(c, 2)
        full[b][:, h * HALF : (h + 1) * HALF] = outs[c].T
    return full.reshape(B, C_IN, 64, 64)


def kernel(**inputs):
    global last_results
    in_maps = prepare_in_maps(**inputs)
    res = run_bass_kernel_spmd(get_nc(), in_maps, core_ids=list(range(8)))
    last_results = res
    return gather_output([r["out"] for r in res.results])


if __name__ == "__main__":
    rng = np.random.default_rng(0)
    ins = {
        "content": rng.standard_normal((B, C_IN, 64, 64), dtype=np.float32),
        "style": rng.standard_normal((B, C_IN, 64, 64), dtype=np.float32),
        "content_key": rng.standard_normal((B, C_KEY, 64, 64), dtype=np.float32),
        "style_key": rng.standard_normal((B, C_KEY, 64, 64), dtype=np.float32),
        "content_mask": rng.integers(0, 2, (B, 1, 64, 64)).astype(np.int32),
        "style_mask": rng.integers(0, 2, (B, 1, 64, 64)).astype(np.int32),
        "Wf": (rng.standard_normal((C_KEY, C_KEY)) * 0.02).astype(np.float32),
        "bf": (rng.standard_normal((C_KEY,)) * 0.02).astype(np.float32),
        "Wg": (rng.standard_normal((C_KEY, C_KEY)) * 0.02).astype(np.float32),
        "bg": (rng.standard_normal((C_KEY,)) * 0.02).astype(np.float32),
        "Wh": (rng.standard_normal((C_IN, C_IN)) * 0.02).astype(np.float32),
        "bh": (rng.standard_normal((C_IN,)) * 0.02).astype(np.float32),
    }
    out = kernel(**ins)
    print("kernel output", out.shape, out.dtype, np.abs(out).mean())


# revision 32
# speedup vs baseline: 1.0483x; 1.0483x over previous
"""Trainium2 Bass kernel for the masked style-attention module.

Shapes (hardcoded): B=4, C_IN=256, C_KEY=448, H=W=64, N=4096.
Sharding: 8 cores = batch (4) x query-row half (2). Each core computes
out[b][:, h*2048:(h+1)*2048] for its (b, h), emitted in [n, c] layout
(host transposes during gather).

Math per core (rows n in its half, all m in 0..4095):
  G_aug[c,m] = Wg@skey + bg  (bias via ones row; row 448 = smi mask)
  Fq_aug[c,n] = Wf@ckey + bf (row 448 = cmneg = -1e15 * content_mask)
  Hv[m,c] = (Wh@style + bh)^T; hv2[m] = [Hv | Hv^2 | 1]  (513 cols)
  T[m,n] = sum_c G_aug[c,m] Fq_aug[c,n]   (mask folds in additively)
  P = exp(T)  (no row-max: |S| < ~40, fp32 psum / bf16 store safe)
  acc[n, 0:513] = sum_m P[m,n] * hv2[m, :]  -> [mean*R | m2*R | R]
  out[n,c] = sqrt(relu(m2-mean^2)) * mvn(content)[n,c] + mean[n,c]

All matmul operands are bf16 (fp32 PSUM accumulate); host pre-quantizes
and pre-rearranges inputs so every DMA is contiguous per partition.
Pipeline: H (style proj) runs first while skey streams in; G, F follow;
attention runs in 8 half-chunks of 256 query rows with a 2-deep exp lag
and the finalize division on the scalar engine so PSUM acc banks free
fast at chunk boundaries.
"""

from contextlib import ExitStack

import numpy as np
import ml_dtypes

import concourse.bass as bass
from concourse import bacc
import concourse.mybir as mybir
import concourse.tile as tile
from concourse.bass_utils import run_bass_kernel_spmd

AF = mybir.ActivationFunctionType
ALU = mybir.AluOpType
F32 = mybir.dt.float32
BF16 = mybir.dt.bfloat16
U32 = mybir.dt.uint32

B, C_IN, C_KEY = 4, 256, 448
N = 4096
HALF = 2048
NEG = -1e15
EPS = 1e-5
CORR = N / (N - 1.0)  # unbiased-variance correction for mvn
MAGIC = 0x5F3759DF

KSZ = [128, 128, 128, 65]  # contraction tiles over 449 (= C_KEY + aug row)
CO448 = [(0, 128), (128, 128), (256, 128), (384, 64)]


def _build():
    nc = bacc.Bacc("TRN2", target_bir_lowering=False)

    skey = nc.dram_tensor("skey", [128, 4, N], BF16, kind="ExternalInput")
    wg = nc.dram_tensor("wg", [128, 4, 448], BF16, kind="ExternalInput")
    ckey = nc.dram_tensor("ckey", [128, 4, HALF], BF16, kind="ExternalInput")
    wf = nc.dram_tensor("wf", [128, 4, 448], BF16, kind="ExternalInput")
    styl = nc.dram_tensor("styl", [128, 2, N], BF16, kind="ExternalInput")
    wh = nc.dram_tensor("wh", [128, 2, 256], BF16, kind="ExternalInput")
    bh = nc.dram_tensor("bh", [1, 256], F32, kind="ExternalInput")
    cont = nc.dram_tensor("cont", [128, 2, N], F32, kind="ExternalInput")
    contT = nc.dram_tensor("contT", [128, 16, 256], BF16, kind="ExternalInput")
    smi = nc.dram_tensor("smi", [1, N], BF16, kind="ExternalInput")
    cmneg = nc.dram_tensor("cmneg", [1, HALF], BF16, kind="ExternalInput")
    ident = nc.dram_tensor("ident", [128, 128], F32, kind="ExternalInput")
    out_d = nc.dram_tensor("out", [HALF, 256], F32, kind="ExternalOutput")

    with tile.TileContext(nc, pool_alloc_mode="queue") as tc:
        with tc.tile_pool(name="persist", bufs=1) as persist:
            g = [
                persist.tile([KSZ[k], N], BF16, tag=f"g{k}", name=f"g{k}")
                for k in range(4)
            ]
            fq = [
                persist.tile([KSZ[k], HALF], BF16, tag=f"fq{k}", name=f"fq{k}")
                for k in range(4)
            ]
            hv2 = persist.tile([128, 32, 513], BF16, tag="hv2", name="hv2")
            aB = persist.tile([128, 256], F32, tag="aB", name="aB")
            bB = persist.tile([128, 256], F32, tag="bB", name="bB")
            ones1 = persist.tile([1, 128], F32, tag="ones1", name="ones1")
            nc.vector.memset(ones1, 1.0)
            nc.vector.memset(hv2[:, :, 512:513], 1.0)
            eps_t = persist.tile([128, 1], F32, tag="eps", name="eps")
            nc.vector.memset(eps_t, EPS)
            mgc = persist.tile([128, 512], U32, tag="mgc", name="mgc")
            nc.vector.memset(mgc, MAGIC)

            _pp = ExitStack()
            pw = _pp.enter_context(tc.tile_pool(name="pw", bufs=1))
            pin = _pp.enter_context(tc.tile_pool(name="pin", bufs=1))
            # DMA issue order = need order: style proj first, then G, then
            # cont (stats run on DVE during G), then F inputs.
            wh_t = pw.tile([128, 2, 256], BF16, tag="wh_t", name="wh_t")
            nc.sync.dma_start(wh_t, wh[:, :, :])
            bh1 = pw.tile([1, 256], F32, tag="bh1", name="bh1")
            nc.sync.dma_start(bh1, bh[:, :])
            st_sb = pin.tile([128, 2, N], BF16, tag="st_sb", name="st_sb")
            nc.sync.dma_start(st_sb[:, :, 0:HALF], styl[:, :, 0:HALF])
            nc.sync.dma_start(st_sb[:, :, HALF:N], styl[:, :, HALF:N])
            wg_t = pw.tile([128, 4, 448], BF16, tag="wg_t", name="wg_t")
            nc.sync.dma_start(wg_t, wg[:, :, :])
            sk_sb = pin.tile([128, 4, N], BF16, tag="sk_sb", name="sk_sb")
            nc.sync.dma_start(sk_sb[:, :, 0:HALF], skey[:, :, 0:HALF])
            nc.sync.dma_start(sk_sb[:, :, HALF:N], skey[:, :, HALF:N])
            cx_sb = pin.tile([128, 2, N], F32, tag="cx_sb", name="cx_sb")
            nc.sync.dma_start(cx_sb[:, 0, :], cont[:, 0, :])
            nc.sync.dma_start(cx_sb[:, 1, :], cont[:, 1, :])
            wf_t = pw.tile([128, 4, 448], BF16, tag="wf_t", name="wf_t")
            nc.sync.dma_start(wf_t, wf[:, :, :])
            ck_sb = pin.tile([128, 4, HALF], BF16, tag="ck_sb", name="ck_sb")
            nc.sync.dma_start(ck_sb, ckey[:, :, :])
            id_sb = pw.tile([128, 128], F32, tag="ident", name="id_sb")
            nc.sync.dma_start(id_sb, ident[:, :])

            # ---- Phase H: hv2[m] = [Wh@style+bh | (.)^2 | 1] ----
            bhb = persist.tile([128, 256], F32, tag="bhb", name="bhb")
            with tc.tile_pool(name="psumH", bufs=4, space="PSUM") as pph:
                pb = pph.tile([128, 256], F32, tag="pbh", name="pbh", bufs=1)
                nc.tensor.matmul(
                    pb, lhsT=ones1[0:1, :], rhs=bh1[0:1, :], start=True, stop=True
                )
                nc.vector.tensor_copy(bhb, pb)
                for mt in range(32):
                    ph = pph.tile([128, 256], F32, tag="ph", name="ph")
                    msl = slice(mt * 128, (mt + 1) * 128)
                    for k in range(2):
                        nc.tensor.matmul(
                            ph,
                            lhsT=st_sb[:, k, msl],
                            rhs=wh_t[:, k, :],
                            start=(k == 0),
                            stop=(k == 1),
                        )
                    nc.vector.tensor_add(hv2[:, mt, 0:256], ph, bhb)
                    nc.vector.tensor_mul(
                        hv2[:, mt, 256:512], hv2[:, mt, 0:256], hv2[:, mt, 0:256]
                    )

            # ---- Phase G: G_aug = Wg @ skey ----
            with tc.tile_pool(name="psumG", bufs=2, space="PSUM") as ppg:
                for half in range(2):
                    for co, (co0, cosz) in enumerate(CO448):
                        pgs = [
                            ppg.tile([128, 512], F32, tag=f"pg{ch}", name=f"pg{ch}")
                            for ch in range(4)
                        ]
                        for k in range(4):
                            for ch in range(4):
                                csl = slice(
                                    half * HALF + ch * 512,
                                    half * HALF + (ch + 1) * 512,
                                )
                                nc.tensor.matmul(
                                    pgs[ch][0:cosz, :],
                                    lhsT=wg_t[0 : KSZ[k], k, co0 : co0 + cosz],
                                    rhs=sk_sb[0 : KSZ[k], k, csl],
                                    start=(k == 0),
                                    stop=(k == 3),
                                )
                        for ch in range(4):
                            csl = slice(
                                half * HALF + ch * 512, half * HALF + (ch + 1) * 512
                            )
                            nc.scalar.copy(g[co][0:cosz, csl], pgs[ch][0:cosz, :])
            nc.sync.dma_start(g[3][64:65, :], smi[:, :])

            # ---- Phase A part 1: mvn stats (DVE/ACT, overlaps G) ----
            a_t = persist.tile([128, 2], F32, tag="a_t", name="a_t")
            b_t = persist.tile([128, 2], F32, tag="b_t", name="b_t")
            pm = _pp.enter_context(tc.tile_pool(name="mvn", bufs=1))
            mvs = []
            for ct in range(2):
                stats = pm.tile([128, 8, 6], F32, tag=f"stats{ct}", name=f"stats{ct}")
                for i in range(8):
                    nc.vector.bn_stats(
                        out=stats[:, i, :], in_=cx_sb[:, ct, i * 512 : (i + 1) * 512]
                    )
                mv = pm.tile([128, 2], F32, tag=f"mv{ct}", name=f"mv{ct}")
                nc.vector.bn_aggr(out=mv, in_=stats)
                mvs.append(mv)
            for ct in range(2):
                lnv = pm.tile([128, 1], F32, tag=f"lnv{ct}", name=f"lnv{ct}")
                nc.scalar.activation(
                    lnv, mvs[ct][:, 1:2], AF.Ln, bias=eps_t[:, 0:1], scale=CORR
                )
                nc.scalar.activation(a_t[:, ct : ct + 1], lnv, AF.Exp, scale=-0.5)
                nc.vector.scalar_tensor_tensor(
                    out=b_t[:, ct : ct + 1],
                    in0=mvs[ct][:, 0:1],
                    scalar=-1.0,
                    in1=a_t[:, ct : ct + 1],
                    op0=ALU.mult,
                    op1=ALU.mult,
                )

            # ---- Phase F: Fq_aug = Wf @ ckey ----
            with tc.tile_pool(name="psumF", bufs=2, space="PSUM") as ppf:
                for co, (co0, cosz) in enumerate(CO448):
                    pfs = [
                        ppf.tile([128, 512], F32, tag=f"pf{ch}", name=f"pf{ch}")
                        for ch in range(4)
                    ]
                    for k in range(4):
                        for ch in range(4):
                            csl = slice(ch * 512, (ch + 1) * 512)
                            nc.tensor.matmul(
                                pfs[ch][0:cosz, :],
                                lhsT=wf_t[0 : KSZ[k], k, co0 : co0 + cosz],
                                rhs=ck_sb[0 : KSZ[k], k, csl],
                                start=(k == 0),
                                stop=(k == 3),
                            )
                    for ch in range(4):
                        csl = slice(ch * 512, (ch + 1) * 512)
                        nc.scalar.copy(fq[co][0:cosz, csl], pfs[ch][0:cosz, :])
            nc.sync.dma_start(fq[3][64:65, :], cmneg[:, :])

            # ---- Phase A part 2: aB/bB broadcast tiles (PE transposes) ----
            with tc.tile_pool(name="psumA", bufs=1, space="PSUM") as ppa:
                rowsA = pm.tile([1, 256], F32, tag="rowsA", name="rowsA")
                rowsB = pm.tile([1, 256], F32, tag="rowsB", name="rowsB")
                for ct in range(2):
                    prow = ppa.tile([1, 128], F32, tag="prow", name="prow", bufs=2)
                    nc.tensor.transpose(prow, a_t[:, ct : ct + 1], id_sb[:, :])
                    nc.vector.tensor_copy(rowsA[0:1, ct * 128 : (ct + 1) * 128], prow)
                    prow2 = ppa.tile([1, 128], F32, tag="prow2", name="prow2", bufs=2)
                    nc.tensor.transpose(prow2, b_t[:, ct : ct + 1], id_sb[:, :])
                    nc.vector.tensor_copy(rowsB[0:1, ct * 128 : (ct + 1) * 128], prow2)
                pab = ppa.tile([128, 256], F32, tag="pab", name="pab", bufs=2)
                nc.tensor.matmul(
                    pab, lhsT=ones1[0:1, :], rhs=rowsA[0:1, :], start=True, stop=True
                )
                nc.vector.tensor_copy(aB, pab)
                pbb = ppa.tile([128, 256], F32, tag="pbb", name="pbb", bufs=2)
                nc.tensor.matmul(
                    pbb, lhsT=ones1[0:1, :], rhs=rowsB[0:1, :], start=True, stop=True
                )
                nc.vector.tensor_copy(bB, pbb)
            _pp.close()  # free projection inputs + stats SBUF

            # ---- Phase D: attention, transposed layout ----
            _dpools = ExitStack()
            fin = _dpools.enter_context(tc.tile_pool(name="fin", bufs=1))
            ptp = _dpools.enter_context(tc.tile_pool(name="ptp", bufs=1))
            with (
                tc.tile_pool(name="ppt", bufs=4, space="PSUM") as ppt,
                tc.tile_pool(name="ppacc", bufs=1, space="PSUM") as ppacc,
            ):
                pend = {}

                def fin_release(st):
                    """Free the acc banks fast: reciprocal + ACT-side divide."""
                    accA, accB = st["accA"], st["accB"]
                    rinvs = []
                    for nt in range(2):
                        rinv = fin.tile([128, 1], F32, tag="rinv", name="rinv", bufs=4)
                        nc.vector.reciprocal(rinv, accB[nt][:, 256:257])
                        rinvs.append(rinv)
                    meanS = fin.tile(
                        [128, 2, 256], F32, tag="meanS", name="meanS", bufs=2
                    )
                    m2S = fin.tile([128, 2, 256], F32, tag="m2S", name="m2S", bufs=2)
                    for nt in range(2):
                        nc.scalar.mul(meanS[:, nt, :], accA[nt], rinvs[nt][:, 0:1])
                        nc.scalar.mul(
                            m2S[:, nt, :], accB[nt][:, 0:256], rinvs[nt][:, 0:1]
                        )
                    st["meanS"], st["m2S"] = meanS, m2S

                def fin_rest(st):
                    """Std + mvn + output, fused over both n-tiles."""
                    hc = st["hc"]
                    meanS, m2S, mvn2 = st["meanS"], st["m2S"], st["mvn2"]
                    mean_f = meanS.rearrange("p a b -> p (a b)")
                    m2_f = m2S.rearrange("p a b -> p (a b)")
                    msq = fin.tile([128, 512], F32, tag="fw", name="msq", bufs=6)
                    nc.vector.tensor_mul(msq, mean_f, mean_f)
                    varp = fin.tile([128, 512], F32, tag="fw", name="varp", bufs=6)
                    nc.vector.scalar_tensor_tensor(
                        out=varp,
                        in0=msq,
                        scalar=-1.0,
                        in1=m2_f,
                        op0=ALU.mult,
                        op1=ALU.add,
                    )
                    varc = fin.tile([128, 512], F32, tag="fw", name="varc", bufs=6)
                    nc.vector.tensor_scalar_max(varc, varp, 1e-20)
                    sh = fin.tile([128, 512], U32, tag="fw", name="sh", bufs=6)
                    nc.vector.tensor_scalar(
                        sh, varc.bitcast(U32), 1, None, ALU.logical_shift_right
                    )
                    y = fin.tile([128, 512], F32, tag="fw", name="y0", bufs=6)
                    nc.vector.tensor_tensor(
                        out=y.bitcast(U32), in0=mgc, in1=sh, op=ALU.subtract
                    )
                    ta = fin.tile([128, 512], F32, tag="fw", name="ta", bufs=6)
                    nc.vector.tensor_mul(ta, y, y)
                    tb = fin.tile([128, 512], F32, tag="fw", name="tb", bufs=6)
                    nc.vector.tensor_mul(tb, ta, varc)
                    tcn = fin.tile([128, 512], F32, tag="fw", name="tcn", bufs=6)
                    nc.vector.tensor_scalar(tcn, tb, -0.5, 1.5, ALU.mult, ALU.add)
                    y2 = fin.tile([128, 512], F32, tag="fw", name="y2", bufs=6)
                    nc.vector.tensor_mul(y2, y, tcn)
                    stdv = fin.tile([128, 512], F32, tag="fw", name="stdv", bufs=6)
                    nc.vector.tensor_mul(stdv, varc, y2)
                    o1 = fin.tile([128, 512], F32, tag="fw", name="o1", bufs=6)
                    nc.vector.tensor_mul(o1, mvn2.rearrange("p a b -> p (a b)"), stdv)
                    o2 = fin.tile([128, 2, 256], F32, tag="o2", name="o2", bufs=2)
                    nc.vector.tensor_add(o2.rearrange("p a b -> p (a b)"), o1, mean_f)
                    nc.sync.dma_start(
                        out_d[hc * 256 : (hc + 1) * 256, :].rearrange(
                            "(t p) c -> p t c", p=128
                        ),
                        o2,
                    )

                for hc in range(8):
                    nsl = slice(hc * 256, (hc + 1) * 256)
                    accA = [
                        ppacc.tile([128, 256], F32, tag=f"accA{nt}", name=f"accA{nt}")
                        for nt in range(2)
                    ]
                    accB = [
                        ppacc.tile([128, 257], F32, tag=f"accB{nt}", name=f"accB{nt}")
                        for nt in range(2)
                    ]
                    # mvn input for this chunk (independent of acc)
                    ct_t = fin.tile([128, 2, 256], BF16, tag="ct_t", name="ct_t", bufs=2)
                    nc.sync.dma_start(ct_t, contT[:, hc * 2 : hc * 2 + 2, :])
                    mvn1 = fin.tile([128, 2, 256], F32, tag="mvn1", name="mvn1", bufs=2)
                    mvn2 = fin.tile([128, 2, 256], F32, tag="mvn2", name="mvn2", bufs=2)
                    for nt in range(2):
                        nc.vector.tensor_mul(mvn1[:, nt, :], ct_t[:, nt, :], aB)
                        nc.vector.tensor_add(mvn2[:, nt, :], mvn1[:, nt, :], bB)

                    def mm2(mt, pt_ap):
                        for nt in range(2):
                            lw = pt_ap[:, nt * 128 : (nt + 1) * 128]
                            nc.tensor.matmul(
                                accA[nt],
                                lhsT=lw,
                                rhs=hv2[:, mt, 0:256],
                                start=(mt == 0),
                                stop=(mt == 31),
                            )
                            nc.tensor.matmul(
                                accB[nt],
                                lhsT=lw,
                                rhs=hv2[:, mt, 256:513],
                                start=(mt == 0),
                                stop=(mt == 31),
                            )

                    prevs = []
                    for mt in range(32):
                        msl = slice(mt * 128, (mt + 1) * 128)
                        tp = ppt.tile([128, 256], F32, tag="tp", name="tp")
                        for k in range(4):
                            nc.tensor.matmul(
                                tp,
                                lhsT=g[k][0 : KSZ[k], msl],
                                rhs=fq[k][0 : KSZ[k], nsl],
                                start=(k == 0),
                                stop=(k == 3),
                            )
                        pt = ptp.tile([128, 256], BF16, tag="pt", name="pt", bufs=4)
                        nc.scalar.activation(pt, tp, AF.Exp)
                        # previous chunk's finalize, interleaved so the ACT
                        # divide lands right after this chunk's first exp
                        if mt == 0 and pend:
                            fin_release(pend)
                        if mt == 2 and pend:
                            fin_rest(pend)
                            pend.clear()
                        prevs.append((mt, pt))
                        if len(prevs) > 2:  # 2-deep lag: exp never blocks PE
                            mm2(*prevs.pop(0))
                    for pr in prevs:
                        mm2(*pr)
                    pend = {"hc": hc, "accA": accA, "accB": accB, "mvn2": mvn2}
                fin_release(pend)
                fin_rest(pend)
            _dpools.close()
    nc.finalize()
    return nc


_nc_cache = None
last_results = None  # BassKernelResults of the most recent run (for test.py)


def _bf16(x):
    return np.asarray(x, dtype=ml_dtypes.bfloat16)


def _pad_k(a449):
    """[449, M] -> [128, 4, M] with k-tiles of 128/128/128/65 (pad to 128)."""
    m = a449.shape[1]
    outp = np.zeros((128, 4, m), a449.dtype)
    for k in range(3):
        outp[:, k, :] = a449[k * 128 : (k + 1) * 128, :]
    outp[0:65, 3, :] = a449[384:449, :]
    return np.ascontiguousarray(outp)


def prepare_in_maps(
    content,
    style,
    content_key,
    style_key,
    content_mask,
    style_mask,
    Wf,
    bf,
    Wg,
    bg,
    Wh,
    bh,
):
    f32 = np.float32
    ones_n = np.ones((1, N), f32)
    ones_h = np.ones((1, HALF), f32)
    wgT = np.concatenate([np.asarray(Wg, f32).T, np.asarray(bg, f32)[None, :]], 0)
    wfT = np.concatenate([np.asarray(Wf, f32).T, np.asarray(bf, f32)[None, :]], 0)
    wg_in = _pad_k(_bf16(wgT))
    wf_in = _pad_k(_bf16(wfT))
    whT = np.asarray(Wh, f32).T.reshape(2, 128, 256).transpose(1, 0, 2)
    wh_in = np.ascontiguousarray(_bf16(whT))
    bh_in = np.ascontiguousarray(np.asarray(bh, f32)[None, :])
    ident_in = np.eye(128, dtype=f32)

    in_maps = []
    for c in range(8):
        b, h = divmod(c, 2)
        hsl = slice(h * HALF, (h + 1) * HALF)
        sk = np.asarray(style_key[b], f32).reshape(C_KEY, N)
        ck = np.asarray(content_key[b], f32).reshape(C_KEY, N)[:, hsl]
        st = np.asarray(style[b], f32).reshape(C_IN, N)
        co = np.asarray(content[b], f32).reshape(C_IN, N)
        smi_in = (np.asarray(style_mask[b], np.int32).reshape(1, N) == 0).astype(f32)
        cm = np.asarray(content_mask[b], np.int32).reshape(N)[hsl]
        cmneg_in = ((cm != 0).astype(f32) * np.float32(NEG))[None, :]
        st_in = _bf16(st).reshape(2, 128, N).transpose(1, 0, 2)
        cont_in = co.reshape(2, 128, N).transpose(1, 0, 2)
        contT_in = _bf16(co[:, hsl].T.reshape(16, 128, 256).transpose(1, 0, 2))
        in_maps.append(
            {
                "skey": _pad_k(_bf16(np.concatenate([sk, ones_n], 0))),
                "wg": wg_in,
                "ckey": _pad_k(_bf16(np.concatenate([ck, ones_h], 0))),
                "wf": wf_in,
                "styl": np.ascontiguousarray(st_in),
                "wh": wh_in,
                "bh": bh_in,
                "cont": np.ascontiguousarray(cont_in),
                "contT": np.ascontiguousarray(contT_in),
                "smi": np.ascontiguousarray(_bf16(smi_in)),
                "cmneg": np.ascontiguousarray(_bf16(cmneg_in)),
                "ident": ident_in,
            }
        )

    return in_maps


def get_nc():
    global _nc_cache
    if _nc_cache is None:
        _nc_cache = _build()
    return _nc_cache


def gather_output(outs):
    full = np.empty((B, C_IN, N), np.float32)
    for c in range(8):
        b, h = divmod(c, 2)
        full[b][:, h * HALF : (h + 1) * HALF] = outs[c].T
    return full.reshape(B, C_IN, 64, 64)


def kernel(**inputs):
    global last_results
    in_maps = prepare_in_maps(**inputs)
    res = run_bass_kernel_spmd(get_nc(), in_maps, core_ids=list(range(8)))
    last_results = res
    return gather_output([r["out"] for r in res.results])


if __name__ == "__main__":
    rng = np.random.default_rng(0)
    ins = {
        "content": rng.standard_normal((B, C_IN, 64, 64), dtype=np.float32),
        "style": rng.standard_normal((B, C_IN, 64, 64), dtype=np.float32),
        "content_key": rng.standard_normal((B, C_KEY, 64, 64), dtype=np.float32),
        "style_key": rng.standard_normal((B, C_KEY, 64, 64), dtype=np.float32),
        "content_mask": rng.integers(0, 2, (B, 1, 64, 64)).astype(np.int32),
        "style_mask": rng.integers(0, 2, (B, 1, 64, 64)).astype(np.int32),
        "Wf": (rng.standard_normal((C_KEY, C_KEY)) * 0.02).astype(np.float32),
        "bf": (rng.standard_normal((C_KEY,)) * 0.02).astype(np.float32),
        "Wg": (rng.standard_normal((C_KEY, C_KEY)) * 0.02).astype(np.float32),
        "bg": (rng.standard_normal((C_KEY,)) * 0.02).astype(np.float32),
        "Wh": (rng.standard_normal((C_IN, C_IN)) * 0.02).astype(np.float32),
        "bh": (rng.standard_normal((C_IN,)) * 0.02).astype(np.float32),
    }
    out = kernel(**ins)
    print("kernel output", out.shape, out.dtype, np.abs(out).mean())
